# revision 6
# baseline (speedup 1.0000x reference)
"""Trainium2 Bass kernel for nn_MBCA (dual-softmax cross-attention gating).

Self-contained: shards batch B=16 across 8 NeuronCores (2 per core),
runs a Tile-scheduled Bass kernel per core, gathers full outputs.

Math (per batch b, head h):
  Xd = drug + pe[b];  Xp = protein + pe[b]        (pe folded on host)
  Qt = (Wq/8)^T-proj of Xd^T  -> [o, t] feature-major (bias folded)
  Kt = Wk^T-proj of Xp^T
  attnT[k, q] = sum_d Kt[d,k] Qt[d,q]             (=(Q K^T)/8 transposed)
  E^T = exp(attnT); colsum[k] via ACT fused accumulate
  PE pass lhsT=[1|1/colsum] over E^T -> [rowsum; M_d][q]
  DVE tensor_tensor_reduce E^T * bcast(1/rowsum)  -> M_p[k]
  drug_attn  = sigmoid(c*(1+M_d) @ W_dp.T + b_dp)  (affine folded into FC)
  protein_attn = sigmoid(c*(1+M_p) @ W_pd.T + b_pd)
  outputs: attn maps, and max over sequence of X*(1+attn).
"""

import math
from contextlib import ExitStack

import ml_dtypes
import numpy as np

import concourse.bass as bass
import concourse.tile as tile
from concourse import bacc, mybir
from concourse.bass_utils import run_bass_kernel_spmd
from concourse.masks import make_identity

N_CORES = 8
B, LD, LP, H, NH, DK = 16, 512, 1024, 512, 8, 64
BPC = B // N_CORES  # batches per core

F32 = mybir.dt.float32
BF16 = mybir.dt.bfloat16
AF = mybir.ActivationFunctionType
ALU = mybir.AluOpType
AX = mybir.AxisListType

_CACHE = {}


def _build():
    """Build + compile the per-core Bass module (SPMD, identical on all cores)."""
    nc = bacc.Bacc("TRN2", target_bir_lowering=False, debug=False, num_devices=N_CORES)

    def inp(name, shape, dt):
        return nc.dram_tensor(name, list(shape), dt, kind="ExternalInput").ap()

    def outp(name, shape, dt):
        return nc.dram_tensor(name, list(shape), dt, kind="ExternalOutput").ap()

    # Inputs (host pre-permuted so every DMA is contiguous per partition)
    xdT_d = inp("xdT", (BPC, 128, 4, LD), BF16)  # (X+pe)^T drug, h-chunked
    xpT_d = inp("xpT", (BPC, 128, 4, LP), BF16)
    xdn_d = inp("xdn", (BPC, 128, 4, H), F32)    # X+pe natural, t-chunked
    xpn_d = inp("xpn", (BPC, 128, 8, H), F32)
    wq_d = inp("wq", (128, 4, 512), BF16)        # (Wq/8)^T  [h-part, h-chunk, o]
    wk_d = inp("wk", (128, 4, 512), BF16)
    bq_d = inp("bq", (128, 4), F32)              # bq/8 per o-chunk column
    bk_d = inp("bk", (128, 4), F32)
    wdp_d = inp("wdp", (9, 512), F32)            # rows 0-7: c*W_dp.T ; row 8: eff bias
    wpd_d = inp("wpd", (9, 512), F32)

    da_d = outp("da", (BPC, 4, 128, H), F32)     # drug_attn  (t-chunked)
    pa_d = outp("pa", (BPC, 8, 128, H), F32)     # protein_attn
    pv_d = outp("pv", (BPC, 8, 128, 1), F32)     # pair vector chunks

    with ExitStack() as ctx:
        tc = ctx.enter_context(tile.TileContext(nc))
        consts = ctx.enter_context(tc.tile_pool(name="consts", bufs=1))
        xpool = ctx.enter_context(tc.tile_pool(name="xpool", bufs=2))
        qkp = ctx.enter_context(tc.tile_pool(name="qkp", bufs=2))
        epool = ctx.enter_context(tc.tile_pool(name="epool", bufs=2))
        small = ctx.enter_context(tc.tile_pool(name="small", bufs=3))
        meanp = ctx.enter_context(tc.tile_pool(name="meanp", bufs=2))
        apool = ctx.enter_context(tc.tile_pool(name="apool", bufs=3))
        psA = ctx.enter_context(tc.tile_pool(name="psA", bufs=2, space="PSUM"))
        psB = ctx.enter_context(tc.tile_pool(name="psB", bufs=2, space="PSUM"))
        psS = ctx.enter_context(tc.tile_pool(name="psS", bufs=2, space="PSUM"))
        psT = ctx.enter_context(tc.tile_pool(name="psT", bufs=2, space="PSUM"))

        # --- constants ---
        wq_sb = consts.tile([128, 4, 512], BF16)
        nc.sync.dma_start(wq_sb, wq_d)
        wk_sb = consts.tile([128, 4, 512], BF16)
        nc.sync.dma_start(wk_sb, wk_d)
        bq_sb = consts.tile([128, 4], F32)
        nc.sync.dma_start(bq_sb, bq_d)
        bk_sb = consts.tile([128, 4], F32)
        nc.sync.dma_start(bk_sb, bk_d)
        wdp_sb = consts.tile([9, 512], F32)
        nc.sync.dma_start(wdp_sb, wdp_d)
        wpd_sb = consts.tile([9, 512], F32)
        nc.sync.dma_start(wpd_sb, wpd_d)
        ident = consts.tile([128, 128], F32)
        make_identity(nc, ident)
        ones_row = consts.tile([1, 128], F32)
        nc.vector.memset(ones_row, 1.0)

        dmeans, pmeans, xdns, xpns = [], [], [], []

        for b in range(BPC):
            # --- load X ---
            xdT_sb = xpool.tile([128, 4, LD], BF16, tag="xdT")
            nc.sync.dma_start(xdT_sb, xdT_d[b])
            xpT_sb = xpool.tile([128, 4, LP], BF16, tag="xpT")
            nc.sync.dma_start(xpT_sb, xpT_d[b])
            xdn_sb = xpool.tile([128, 4, H], F32, tag="xdn")
            nc.sync.dma_start(xdn_sb, xdn_d[b])
            xpn_sb = xpool.tile([128, 8, H], F32, tag="xpn")
            nc.sync.dma_start(xpn_sb, xpn_d[b])
            xdns.append(xdn_sb)
            xpns.append(xpn_sb)

            # --- projections: Qt [o, t] and Kt [o, k], feature-major bf16 ---
            qt_sb = qkp.tile([128, 4, LD], BF16, tag="qt")
            for m in range(4):
                ps = psB.tile([128, 512], F32, tag="mm")
                for c in range(4):
                    nc.tensor.matmul(
                        ps,
                        wq_sb[:, c, 128 * m : 128 * (m + 1)],
                        xdT_sb[:, c],
                        start=(c == 0),
                        stop=(c == 3),
                    )
                nc.vector.tensor_scalar_add(qt_sb[:, m], ps, bq_sb[:, m : m + 1])
            kt_sb = qkp.tile([128, 4, LP], BF16, tag="kt")
            for m in range(4):
                for half in range(2):
                    ps = psB.tile([128, 512], F32, tag="mm")
                    for c in range(4):
                        nc.tensor.matmul(
                            ps,
                            wk_sb[:, c, 128 * m : 128 * (m + 1)],
                            xpT_sb[:, c, 512 * half : 512 * (half + 1)],
                            start=(c == 0),
                            stop=(c == 3),
                        )
                    nc.vector.tensor_scalar_add(
                        kt_sb[:, m, 512 * half : 512 * (half + 1)],
                        ps,
                        bk_sb[:, m : m + 1],
                    )

            # --- per-b mean accumulators ---
            # memset whole tile to 1.0: rows 0-7 are overwritten per head,
            # row 8 stays 1.0 (the "ones" row of the augmented FC lhsT).
            dmean_sb = meanp.tile([9, LD], F32, tag="dmean")
            nc.vector.memset(dmean_sb, 1.0)
            pmean_sb = meanp.tile([9, LP], F32, tag="pmean")
            nc.vector.memset(pmean_sb, 1.0)
            mpkh_sb = meanp.tile([128, 64], F32, tag="mpkh")  # [k-part, 8c*8h]
            dmeans.append(dmean_sb)
            pmeans.append(pmean_sb)

            # --- heads ---
            for h in range(NH):
                po = 64 * (h % 2)
                m = h // 2
                qt_h = qt_sb[po : po + 64, m]  # [64, LD]
                et = epool.tile([128, 8, 512], BF16, tag="et")
                colsum = small.tile([128, 8], F32, tag="colsum")
                for c in range(8):
                    psat = psA.tile([128, 512], F32, tag="attn")
                    nc.tensor.matmul(
                        psat,
                        kt_sb[po : po + 64, m, 128 * c : 128 * (c + 1)],
                        qt_h,
                        start=True,
                        stop=True,
                    )
                    nc.scalar.activation(
                        et[:, c], psat, AF.Exp, accum_out=colsum[:, c : c + 1]
                    )
                # lhsT [ones | 1/colsum] interleaved, bf16
                packed = small.tile([128, 16], BF16, tag="packed")
                nc.vector.memset(packed, 1.0)
                pk_v = packed.rearrange("p (c two) -> p c two", two=2)
                with nc.allow_low_precision("bf16 invcol is within tolerance"):
                    nc.vector.reciprocal(pk_v[:, :, 1], colsum)
                ps4 = psS.tile([2, 512], F32, tag="p4")
                for c in range(8):
                    nc.tensor.matmul(
                        ps4,
                        packed[:, 2 * c : 2 * (c + 1)],
                        et[:, c],
                        start=(c == 0),
                        stop=(c == 7),
                    )
                # rowsum -> PE broadcast -> reciprocal; M_d row -> dmean
                rs_sb = small.tile([2, 512], F32, tag="rs")
                nc.vector.tensor_copy(rs_sb, ps4)
                nc.sync.dma_start(dmean_sb[h : h + 1, :], rs_sb[1:2, :])
                psbr = psB.tile([128, 512], F32, tag="mm")
                nc.tensor.matmul(psbr, ones_row, rs_sb[0:1, :], start=True, stop=True)
                invr = small.tile([128, 512], BF16, tag="invr")
                with nc.allow_low_precision("bf16 invrow is within tolerance"):
                    nc.vector.reciprocal(invr, psbr)
                # fused multiply + free-dim sum: accum = sum(E^T * invrow)
                # (tensor_tensor_reduce crashes NRT on this stack; stt works)
                scratch = small.tile([128, 512], BF16, tag="scratch")
                for c in range(8):
                    nc.vector.scalar_tensor_tensor(
                        scratch,
                        et[:, c],
                        1.0,
                        invr,
                        ALU.mult,
                        ALU.mult,
                        accum_out=mpkh_sb[:, 8 * c + h : 8 * c + h + 1],
                    )

            # --- assemble pmean rows via PE transpose of [128, 8] blocks ---
            for c in range(8):
                pst = psT.tile([8, 128], F32, tag="t")
                nc.tensor.transpose(pst, mpkh_sb[:, 8 * c : 8 * (c + 1)], ident)
                nc.vector.tensor_copy(pmean_sb[0:8, 128 * c : 128 * (c + 1)], pst)

        # --- FC + sigmoid + gating + sequence max (both batches) ---
        for b in range(BPC):
            dmx = None
            for m in range(4):
                psfc = psB.tile([128, 512], F32, tag="mm")
                nc.tensor.matmul(
                    psfc,
                    dmeans[b][:, 128 * m : 128 * (m + 1)],
                    wdp_sb,
                    start=True,
                    stop=True,
                )
                a_t = apool.tile([128, 512], F32, tag="a")
                nc.scalar.activation(a_t, psfc, AF.Sigmoid)
                nc.sync.dma_start(da_d[b, m], a_t)
                g_t = apool.tile([128, 512], F32, tag="g")
                nc.vector.scalar_tensor_tensor(
                    g_t, a_t, 1.0, xdns[b][:, m], ALU.add, ALU.mult
                )
                if dmx is None:
                    dmx = g_t
                else:
                    nmx = apool.tile([128, 512], F32, tag="mx")
                    nc.vector.tensor_max(nmx, dmx, g_t)
                    dmx = nmx
            for c4 in range(4):
                pst = psT.tile([128, 128], F32, tag="t")
                nc.tensor.transpose(pst, dmx[:, 128 * c4 : 128 * (c4 + 1)], ident)
                vx = small.tile([128, 1], F32, tag="vx")
                nc.vector.tensor_reduce(vx, pst, AX.X, ALU.max)
                nc.sync.dma_start(pv_d[b, c4], vx)

            pmx = None
            for c in range(8):
                psfc = psB.tile([128, 512], F32, tag="mm")
                nc.tensor.matmul(
                    psfc,
                    pmeans[b][:, 128 * c : 128 * (c + 1)],
                    wpd_sb,
                    start=True,
                    stop=True,
                )
                a_t = apool.tile([128, 512], F32, tag="a")
                nc.scalar.activation(a_t, psfc, AF.Sigmoid)
                nc.sync.dma_start(pa_d[b, c], a_t)
                g_t = apool.tile([128, 512], F32, tag="g")
                nc.vector.scalar_tensor_tensor(
                    g_t, a_t, 1.0, xpns[b][:, c], ALU.add, ALU.mult
                )
                if pmx is None:
                    pmx = g_t
                else:
                    nmx = apool.tile([128, 512], F32, tag="mx")
                    nc.vector.tensor_max(nmx, pmx, g_t)
                    pmx = nmx
            for c4 in range(4):
                pst = psT.tile([128, 128], F32, tag="t")
                nc.tensor.transpose(pst, pmx[:, 128 * c4 : 128 * (c4 + 1)], ident)
                vx = small.tile([128, 1], F32, tag="vx")
                nc.vector.tensor_reduce(vx, pst, AX.X, ALU.max)
                nc.sync.dma_start(pv_d[b, 4 + c4], vx)

    nc.compile()
    return nc


def _sin_pe(max_len, d_model):
    pos = np.arange(max_len, dtype=np.float32)[:, None]
    div = np.exp(
        np.arange(0, d_model, 2, dtype=np.float32) * (-math.log(10000.0) / d_model)
    )
    pe = np.zeros((max_len, d_model), dtype=np.float32)
    pe[:, 0::2] = np.sin(pos * div)
    pe[:, 1::2] = np.cos(pos * div)
    return pe


def get_module():
    if "nc" not in _CACHE:
        _CACHE["nc"] = _build()
    return _CACHE["nc"]


def _prep_inputs(drug, protein, Wq, bq, Wk, bk, W_dp, b_dp, W_pd, b_pd):
    """Host-side shard prep: pe-add, transposes, weight folds. Returns in_maps."""
    bf16 = ml_dtypes.bfloat16
    drug = np.asarray(drug, np.float32)
    protein = np.asarray(protein, np.float32)
    Wq = np.asarray(Wq, np.float32)
    bq = np.asarray(bq, np.float32)
    Wk = np.asarray(Wk, np.float32)
    bk = np.asarray(bk, np.float32)
    W_dp = np.asarray(W_dp, np.float32)
    b_dp = np.asarray(b_dp, np.float32)
    W_pd = np.asarray(W_pd, np.float32)
    b_pd = np.asarray(b_pd, np.float32)

    pe = _sin_pe(B, H)  # rows 0..B-1 identical for both tables
    xd = drug + pe[:, None, :]  # [B, LD, H]
    xp = protein + pe[:, None, :]  # [B, LP, H]

    # natural, t-chunked: [B, 128, nchunk, H]
    xdn = np.ascontiguousarray(xd.reshape(B, 4, 128, H).transpose(0, 2, 1, 3))
    xpn = np.ascontiguousarray(xp.reshape(B, 8, 128, H).transpose(0, 2, 1, 3))
    # transposed, h-chunked: [B, 128, 4, L]
    xdT = np.ascontiguousarray(
        xd.transpose(0, 2, 1).reshape(B, 4, 128, LD).transpose(0, 2, 1, 3)
    ).astype(bf16)
    xpT = np.ascontiguousarray(
        xp.transpose(0, 2, 1).reshape(B, 4, 128, LP).transpose(0, 2, 1, 3)
    ).astype(bf16)

    wq_h = np.ascontiguousarray(
        (Wq / 8.0).T.reshape(4, 128, 512).transpose(1, 0, 2)
    ).astype(bf16)
    wk_h = np.ascontiguousarray(Wk.T.reshape(4, 128, 512).transpose(1, 0, 2)).astype(
        bf16
    )
    bq_h = np.ascontiguousarray((bq / 8.0).reshape(4, 128).T)
    bk_h = np.ascontiguousarray(bk.reshape(4, 128).T)
    cdp = 0.5 / LP
    cpd = 0.5 / LD
    wdp_h = np.concatenate(
        [cdp * W_dp.T, (b_dp + cdp * W_dp.sum(axis=1))[None, :]], axis=0
    ).astype(np.float32)  # [9, 512]
    wpd_h = np.concatenate(
        [cpd * W_pd.T, (b_pd + cpd * W_pd.sum(axis=1))[None, :]], axis=0
    ).astype(np.float32)

    in_maps = []
    for i in range(N_CORES):
        s = slice(i * BPC, (i + 1) * BPC)
        in_maps.append(
            {
                "xdT": xdT[s],
                "xpT": xpT[s],
                "xdn": xdn[s],
                "xpn": xpn[s],
                "wq": wq_h,
                "wk": wk_h,
                "bq": bq_h,
                "bk": bk_h,
                "wdp": wdp_h,
                "wpd": wpd_h,
            }
        )
    return in_maps


def kernel(drug, protein, Wq, bq, Wk, bk, W_dp, b_dp, W_pd, b_pd):
    nc = get_module()
    in_maps = _prep_inputs(drug, protein, Wq, bq, Wk, bk, W_dp, b_dp, W_pd, b_pd)
    res = run_bass_kernel_spmd(nc, in_maps, core_ids=list(range(N_CORES)))
    da = np.concatenate([res.results[i]["da"] for i in range(N_CORES)], axis=0)
    pa = np.concatenate([res.results[i]["pa"] for i in range(N_CORES)], axis=0)
    pv = np.concatenate([res.results[i]["pv"] for i in range(N_CORES)], axis=0)
    drug_attn = np.ascontiguousarray(da.reshape(B, LD, H))
    protein_attn = np.ascontiguousarray(pa.reshape(B, LP, H))
    pair = np.ascontiguousarray(pv.reshape(B, 2 * H))
    return pair, (drug_attn, protein_attn)


# revision 24
# speedup vs baseline: 1.2836x; 1.2836x over previous
"""Trainium2 Bass kernel for nn_MBCA (dual-softmax cross-attention gating).

Self-contained: shards batch B=16 across 8 NeuronCores (2 per core),
runs a Tile-scheduled Bass kernel per core, gathers full outputs.

Math (per batch b, head h):
  Xd = drug + pe[b];  Xp = protein + pe[b]        (pe folded on host)
  Qt = (Wq/8)^T-proj of Xd^T  -> [o, t] feature-major (bias folded)
  Kt = Wk^T-proj of Xp^T
  attnT[k, q] = sum_d Kt[d,k] Qt[d,q]             (=(Q K^T)/8 transposed)
  E^T = exp(attnT); colsum[k] via ACT fused accumulate
  PE pass lhsT=[1|1/colsum] over E^T -> [rowsum; M_d][q]
  DVE stt E^T * bcast(1/rowsum) with accum        -> M_p[k]
  drug_attn  = sigmoid(c*(1+M_d) @ W_dp.T + b_dp)  (affine folded into FC)
  protein_attn = sigmoid(c*(1+M_p) @ W_pd.T + b_pd)
  outputs: attn maps, and max over sequence of X*(1+attn).
"""

import math
from contextlib import ExitStack

import ml_dtypes
import numpy as np

import concourse.bass as bass
import concourse.tile as tile
from concourse.tile import add_dep_helper
from concourse import bacc, bass_isa, mybir
from concourse.bass_utils import run_bass_kernel_spmd
from concourse.masks import make_identity

N_CORES = 8
B, LD, LP, H, NH, DK = 16, 512, 1024, 512, 8, 64
BPC = B // N_CORES  # batches per core

F32 = mybir.dt.float32
BF16 = mybir.dt.bfloat16
AF = mybir.ActivationFunctionType
ALU = mybir.AluOpType
AX = mybir.AxisListType

_CACHE = {}


def _build():
    """Build + compile the per-core Bass module (SPMD, identical on all cores)."""
    nc = bacc.Bacc("TRN2", target_bir_lowering=False, debug=False, num_devices=N_CORES)

    def inp(name, shape, dt):
        return nc.dram_tensor(name, list(shape), dt, kind="ExternalInput").ap()

    def outp(name, shape, dt):
        return nc.dram_tensor(name, list(shape), dt, kind="ExternalOutput").ap()

    # Inputs (host pre-permuted so every DMA is contiguous per partition)
    xdT_d = inp("xdT", (BPC, 128, 4, LD), BF16)  # (X+pe)^T drug, h-chunked
    xpT_d = inp("xpT", (BPC, 128, 4, LP), BF16)
    xdn_d = inp("xdn", (BPC, 128, 4, H), F32)    # X+pe natural, t-chunked
    xpn_d = inp("xpn", (BPC, 128, 8, H), F32)
    wq_d = inp("wq", (128, 4, 512), BF16)        # (Wq/8)^T  [h-part, h-chunk, o]
    wk_d = inp("wk", (128, 4, 512), BF16)
    bq_d = inp("bq", (128, 4), F32)              # bq/8 per o-chunk column
    bk_d = inp("bk", (128, 4), F32)
    wdp_d = inp("wdp", (9, 512), BF16)           # rows 0-7: c*W_dp.T ; row 8: eff bias
    wpd_d = inp("wpd", (9, 512), BF16)

    da_d = outp("da", (BPC, 4, 128, H), F32)     # drug_attn  (t-chunked)
    pa_d = outp("pa", (BPC, 8, 128, H), F32)     # protein_attn
    pv_d = outp("pv", (BPC, 2, 1, 512), F32)     # [drug_v; protein_v]

    with ExitStack() as ctx:
        tc = ctx.enter_context(tile.TileContext(nc))
        consts = ctx.enter_context(tc.tile_pool(name="consts", bufs=1))
        xpool = ctx.enter_context(tc.tile_pool(name="xpool", bufs=2))
        qkp = ctx.enter_context(tc.tile_pool(name="qkp", bufs=2))
        epool = ctx.enter_context(tc.tile_pool(name="epool", bufs=4))
        small = ctx.enter_context(tc.tile_pool(name="small", bufs=6))
        meanp = ctx.enter_context(tc.tile_pool(name="meanp", bufs=2))
        apool = ctx.enter_context(tc.tile_pool(name="apool", bufs=4))
        psA = ctx.enter_context(tc.tile_pool(name="psA", bufs=2, space="PSUM"))
        psB = ctx.enter_context(tc.tile_pool(name="psB", bufs=2, space="PSUM"))
        psS = ctx.enter_context(tc.tile_pool(name="psS", bufs=2, space="PSUM"))
        psT = ctx.enter_context(tc.tile_pool(name="psT", bufs=2, space="PSUM"))

        # --- constants ---
        wq_sb = consts.tile([128, 4, 512], BF16)
        nc.sync.dma_start(wq_sb, wq_d)
        wk_sb = consts.tile([128, 4, 512], BF16)
        nc.sync.dma_start(wk_sb, wk_d)
        bq_sb = consts.tile([128, 4], F32)
        nc.sync.dma_start(bq_sb, bq_d)
        bk_sb = consts.tile([128, 4], F32)
        nc.sync.dma_start(bk_sb, bk_d)
        wdp_sb = consts.tile([9, 512], BF16)
        nc.sync.dma_start(wdp_sb, wdp_d)
        wpd_sb = consts.tile([9, 512], BF16)
        nc.sync.dma_start(wpd_sb, wpd_d)
        ident = consts.tile([128, 128], F32)
        make_identity(nc, ident)

        # --- per-b state ---
        qts, kts, xdns, xpns, dmeans, pmeans, mpkhs = [], [], [], [], [], [], []
        xdTs, xpTs = [], []
        last_exp = [None]  # last exp instruction; sigmoids are pinned after it

        # --- loads for both batches (DMAs queue up front) ---
        for b in range(BPC):
            xdT_sb = xpool.tile([128, 4, LD], BF16, tag="xdT")
            nc.sync.dma_start(xdT_sb, xdT_d[b])
            xpT_sb = xpool.tile([128, 4, LP], BF16, tag="xpT")
            nc.sync.dma_start(xpT_sb, xpT_d[b])
            xdn_sb = xpool.tile([128, 4, H], F32, tag="xdn")
            nc.sync.dma_start(xdn_sb, xdn_d[b])
            xpn_sb = xpool.tile([128, 8, H], F32, tag="xpn")
            nc.sync.dma_start(xpn_sb, xpn_d[b])
            xdns.append(xdn_sb)
            xpns.append(xpn_sb)
            xdTs.append(xdT_sb)
            xpTs.append(xpT_sb)
            # memset whole tile to 1.0: rows 0-7 are overwritten per head,
            # row 8 stays 1.0 (the "ones" row of the augmented FC lhsT).
            dmean_sb = meanp.tile([9, LD], BF16, tag="dmean")
            nc.vector.memset(dmean_sb, 1.0)
            pmean_sb = meanp.tile([9, LP], BF16, tag="pmean")
            nc.vector.memset(pmean_sb, 1.0)
            mpkh_sb = meanp.tile([128, 64], F32, tag="mpkh")  # [k-part, 8c*8h]
            dmeans.append(dmean_sb)
            pmeans.append(pmean_sb)
            mpkhs.append(mpkh_sb)
            qts.append(None)
            kts.append(None)

        def emit_proj_chunk(b, m):
            """Q and K projections for one o-chunk m (heads 2m, 2m+1)."""
            xdT_sb, xpT_sb = xdTs[b], xpTs[b]
            ps = psB.tile([128, 512], F32, tag="mm")
            for c in range(4):
                nc.tensor.matmul(
                    ps,
                    wq_sb[:, c, 128 * m : 128 * (m + 1)],
                    xdT_sb[:, c],
                    start=(c == 0),
                    stop=(c == 3),
                )
            nc.vector.tensor_scalar_add(qts[b][:, m], ps, bq_sb[:, m : m + 1])
            for half in range(2):
                ps = psB.tile([128, 512], F32, tag="mm")
                for c in range(4):
                    nc.tensor.matmul(
                        ps,
                        wk_sb[:, c, 128 * m : 128 * (m + 1)],
                        xpT_sb[:, c, 512 * half : 512 * (half + 1)],
                        start=(c == 0),
                        stop=(c == 3),
                    )
                nc.vector.tensor_scalar_add(
                    kts[b][:, m, 512 * half : 512 * (half + 1)],
                    ps,
                    bk_sb[:, m : m + 1],
                )

        def alloc_qk(b):
            qts[b] = qkp.tile([128, 4, LD], BF16, tag="qt", name=f"qt{b}")
            kts[b] = qkp.tile([128, 4, LP], BF16, tag="kt", name=f"kt{b}")

        def front(b, h):
            """attn + exp(+colsum accum) + packed [1|1/colsum] lhsT."""
            po = 64 * (h % 2)
            m = h // 2
            qt_h = qts[b][po : po + 64, m]  # [64, LD]
            et = epool.tile([128, 8, 512], BF16, tag="et")
            colsum = small.tile([128, 8], F32, tag="colsum")
            for c in range(8):
                psat = psA.tile([128, 512], F32, tag="attn")
                nc.tensor.matmul(
                    psat,
                    kts[b][po : po + 64, m, 128 * c : 128 * (c + 1)],
                    qt_h,
                    start=True,
                    stop=True,
                )
                last_exp[0] = nc.scalar.activation(
                    et[:, c], psat, AF.Exp, accum_out=colsum[:, c : c + 1]
                )
            packed = small.tile([128, 16], BF16, tag="packed")
            nc.vector.memset(packed, 1.0)
            pk_v = packed.rearrange("p (c two) -> p c two", two=2)
            with nc.allow_low_precision("bf16 invcol is within tolerance"):
                nc.vector.reciprocal(pk_v[:, :, 1], colsum)
            return (b, h, et, packed)

        def tail(state):
            """pass4 -> [rowsum; M_d]; rowsum bcast; stt accum -> M_p."""
            b, h, et, packed = state
            ps4 = psS.tile([2, 512], F32, tag="p4")
            for c in range(8):
                nc.tensor.matmul(
                    ps4,
                    packed[:, 2 * c : 2 * (c + 1)],
                    et[:, c],
                    start=(c == 0),
                    stop=(c == 7),
                )
            # rowsum+M_d -> SBUF (bf16) on ACT (Copy is in every table
            # set, so no ACT table reload); M_d row -> dmean via DMA
            rs_sb = small.tile([2, 512], BF16, tag="rs")
            nc.scalar.activation(rs_sb, ps4, AF.Copy)
            nc.sync.dma_start(dmeans[b][h : h + 1, :], rs_sb[1:2, :])
            # 1/rowsum row, broadcast to all partitions on GpSimd
            invrow_row = small.tile([1, 512], BF16, tag="invrow_row")
            with nc.allow_low_precision("bf16 invrow is within tolerance"):
                nc.vector.reciprocal(invrow_row, rs_sb[0:1, :])
            invr = small.tile([128, 512], BF16, tag="invr")
            nc.gpsimd.partition_broadcast(invr, invrow_row)
            # fused multiply + free-dim sum: accum = sum(E^T * invrow) = M_p
            # (tensor_tensor_reduce crashes NRT on this stack; stt works)
            scratch = small.tile([128, 512], BF16, tag="scratch")
            for c in range(8):
                nc.vector.scalar_tensor_tensor(
                    scratch,
                    et[:, c],
                    1.0,
                    invr,
                    ALU.mult,
                    ALU.mult,
                    accum_out=mpkhs[b][:, 8 * c + h : 8 * c + h + 1],
                )

        def emit_pmean_transposes(b):
            # assemble pmean rows via PE transpose of the [128, 8] M_p blocks
            for c in range(8):
                pst = psT.tile([8, 128], F32, tag="t")
                nc.tensor.transpose(pst, mpkhs[b][:, 8 * c : 8 * (c + 1)], ident)
                nc.vector.tensor_copy(pmeans[b][0:8, 128 * c : 128 * (c + 1)], pst)

        # Sigmoids chain one-after-another so they run as one ACT burst
        # instead of scattering between exps (each scatter costs two
        # ~1.3us ACT table reloads).
        prev_sig = [None]

        def chain_sigmoid(sg):
            if prev_sig[0] is not None:
                add_dep_helper(sg.ins, prev_sig[0].ins, sync=False,
                               reason="sigmoid burst ordering")
            prev_sig[0] = sg

        # --- FC + sigmoid + gating + sequence max for one batch ---
        def emit_fc(b):
            dmx = None
            for m in range(4):
                psfc = psB.tile([128, 512], F32, tag="mm")
                nc.tensor.matmul(
                    psfc,
                    dmeans[b][:, 128 * m : 128 * (m + 1)],
                    wdp_sb,
                    start=True,
                    stop=True,
                )
                a_t = apool.tile([128, 512], F32, tag="a")
                sg = nc.scalar.activation(a_t, psfc, AF.Sigmoid)
                chain_sigmoid(sg)
                nc.sync.dma_start(da_d[b, m], a_t)
                g_t = apool.tile([128, 512], BF16, tag="g")
                nc.vector.scalar_tensor_tensor(
                    g_t, a_t, 1.0, xdns[b][:, m], ALU.add, ALU.mult
                )
                if dmx is None:
                    dmx = g_t
                else:
                    nmx = apool.tile([128, 512], BF16, tag="mx")
                    nc.vector.tensor_max(nmx, dmx, g_t)
                    dmx = nmx
            var = apool.tile([128, 512], F32, tag="var")
            nc.gpsimd.partition_all_reduce(var, dmx, 128, bass_isa.ReduceOp.max)
            nc.sync.dma_start(pv_d[b, 0], var[0:1, :])

            pmx = None
            for c in range(8):
                psfc = psB.tile([128, 512], F32, tag="mm")
                nc.tensor.matmul(
                    psfc,
                    pmeans[b][:, 128 * c : 128 * (c + 1)],
                    wpd_sb,
                    start=True,
                    stop=True,
                )
                a_t = apool.tile([128, 512], F32, tag="a")
                sg = nc.scalar.activation(a_t, psfc, AF.Sigmoid)
                chain_sigmoid(sg)
                nc.sync.dma_start(pa_d[b, c], a_t)
                g_t = apool.tile([128, 512], BF16, tag="g")
                nc.vector.scalar_tensor_tensor(
                    g_t, a_t, 1.0, xpns[b][:, c], ALU.add, ALU.mult
                )
                if pmx is None:
                    pmx = g_t
                else:
                    nmx = apool.tile([128, 512], BF16, tag="mx")
                    nc.vector.tensor_max(nmx, pmx, g_t)
                    pmx = nmx
            var2 = apool.tile([128, 512], F32, tag="var")
            nc.gpsimd.partition_all_reduce(var2, pmx, 128, bass_isa.ReduceOp.max)
            nc.sync.dma_start(pv_d[b, 1], var2[0:1, :])

        # --- emission order ---
        # b0: proj chunk m interleaves with heads 2(m-1)/2(m-1)+1 so the
        # first attn starts ~7us in; b1 proj slots in at b0 head 3 (its
        # DMAs are long done); b0's whole FC/gating/output phase overlaps
        # b1's head phase; tails lag fronts by one head throughout.
        alloc_qk(0)
        emit_proj_chunk(0, 0)
        pending = None
        for b in range(BPC):
            for h in range(NH):
                if b == 0 and h in (0, 2, 4):
                    emit_proj_chunk(0, h // 2 + 1)
                if b == 0 and h == 3 and BPC > 1:
                    alloc_qk(1)
                    for m in range(4):
                        emit_proj_chunk(1, m)
                st = front(b, h)
                if pending is not None:
                    pb = pending[0]
                    tail(pending)
                    if pending[1] == NH - 1:  # last tail of batch pb done
                        emit_pmean_transposes(pb)
                pending = st
        pb = pending[0]
        tail(pending)
        emit_pmean_transposes(pb)
        for bb in range(BPC):
            emit_fc(bb)

    nc.compile()
    return nc


def _sin_pe(max_len, d_model):
    pos = np.arange(max_len, dtype=np.float32)[:, None]
    div = np.exp(
        np.arange(0, d_model, 2, dtype=np.float32) * (-math.log(10000.0) / d_model)
    )
    pe = np.zeros((max_len, d_model), dtype=np.float32)
    pe[:, 0::2] = np.sin(pos * div)
    pe[:, 1::2] = np.cos(pos * div)
    return pe


def get_module():
    if "nc" not in _CACHE:
        _CACHE["nc"] = _build()
    return _CACHE["nc"]


def _prep_inputs(drug, protein, Wq, bq, Wk, bk, W_dp, b_dp, W_pd, b_pd):
    """Host-side shard prep: pe-add, transposes, weight folds. Returns in_maps."""
    bf16 = ml_dtypes.bfloat16
    drug = np.asarray(drug, np.float32)
    protein = np.asarray(protein, np.float32)
    Wq = np.asarray(Wq, np.float32)
    bq = np.asarray(bq, np.float32)
    Wk = np.asarray(Wk, np.float32)
    bk = np.asarray(bk, np.float32)
    W_dp = np.asarray(W_dp, np.float32)
    b_dp = np.asarray(b_dp, np.float32)
    W_pd = np.asarray(W_pd, np.float32)
    b_pd = np.asarray(b_pd, np.float32)

    pe = _sin_pe(B, H)  # rows 0..B-1 identical for both tables
    xd = drug + pe[:, None, :]  # [B, LD, H]
    xp = protein + pe[:, None, :]  # [B, LP, H]

    # natural, t-chunked: [B, 128, nchunk, H]
    xdn = np.ascontiguousarray(xd.reshape(B, 4, 128, H).transpose(0, 2, 1, 3))
    xpn = np.ascontiguousarray(xp.reshape(B, 8, 128, H).transpose(0, 2, 1, 3))
    # transposed, h-chunked: [B, 128, 4, L]
    xdT = np.ascontiguousarray(
        xd.transpose(0, 2, 1).reshape(B, 4, 128, LD).transpose(0, 2, 1, 3)
    ).astype(bf16)
    xpT = np.ascontiguousarray(
        xp.transpose(0, 2, 1).reshape(B, 4, 128, LP).transpose(0, 2, 1, 3)
    ).astype(bf16)

    wq_h = np.ascontiguousarray(
        (Wq / 8.0).T.reshape(4, 128, 512).transpose(1, 0, 2)
    ).astype(bf16)
    wk_h = np.ascontiguousarray(Wk.T.reshape(4, 128, 512).transpose(1, 0, 2)).astype(
        bf16
    )
    bq_h = np.ascontiguousarray((bq / 8.0).reshape(4, 128).T)
    bk_h = np.ascontiguousarray(bk.reshape(4, 128).T)
    cdp = 0.5 / LP
    cpd = 0.5 / LD
    wdp_h = np.concatenate(
        [cdp * W_dp.T, (b_dp + cdp * W_dp.sum(axis=1))[None, :]], axis=0
    ).astype(bf16)  # [9, 512]
    wpd_h = np.concatenate(
        [cpd * W_pd.T, (b_pd + cpd * W_pd.sum(axis=1))[None, :]], axis=0
    ).astype(bf16)

    in_maps = []
    for i in range(N_CORES):
        s = slice(i * BPC, (i + 1) * BPC)
        in_maps.append(
            {
                "xdT": xdT[s],
                "xpT": xpT[s],
                "xdn": xdn[s],
                "xpn": xpn[s],
                "wq": wq_h,
                "wk": wk_h,
                "bq": bq_h,
                "bk": bk_h,
                "wdp": wdp_h,
                "wpd": wpd_h,
            }
        )
    return in_maps


def kernel(drug, protein, Wq, bq, Wk, bk, W_dp, b_dp, W_pd, b_pd):
    nc = get_module()
    in_maps = _prep_inputs(drug, protein, Wq, bq, Wk, bk, W_dp, b_dp, W_pd, b_pd)
    res = run_bass_kernel_spmd(nc, in_maps, core_ids=list(range(N_CORES)))
    da = np.concatenate([res.results[i]["da"] for i in range(N_CORES)], axis=0)
    pa = np.concatenate([res.results[i]["pa"] for i in range(N_CORES)], axis=0)
    pv = np.concatenate([res.results[i]["pv"] for i in range(N_CORES)], axis=0)
    drug_attn = np.ascontiguousarray(da.reshape(B, LD, H))
    protein_attn = np.ascontiguousarray(pa.reshape(B, LP, H))
    pair = np.ascontiguousarray(pv.reshape(B, 2 * H))
    return pair, (drug_attn, protein_attn)
